# revision 2
# baseline (speedup 1.0000x reference)
"""Trainium2 Bass kernel for nn_AutoRegressive (dense transformer decoder), v2.

Model: B=4 packed text+audio sequences, L=768, D=1024, 16 heads, DFF=4096,
6 norm-first decoder layers (self-attn w/ prefix-LM mask, cross-attn to the
packed embedding, FFN), weight-tied audio head. fp32 inputs/outputs.

Sharding: DP4 x TP2 over 8 cores. Core pair (2i, 2i+1) owns batch item i;
within a pair the 16 heads split 8+8 and DFF splits 2048+2048. Three pair-
AllReduces per layer; CA K/V projections are scheduled into AR shadows.

v2 design vs baseline:
- all weights pre-transposed + bf16 on host (no PE transposes, no ACT copies)
- activations bf16 (residual stream f32), matmuls bf16 @ 1 cyc/row
- embedding + positional encoding gathered/packed on host -> x0 DMA
- V projected token-major directly (stationary = activation tile), so AV
  stationary needs no on-device transpose; ones column gives softmax denom
- SA score tiles that are fully masked are skipped (12 of 18 (tk,chunk))
- mask applied multiplicatively AFTER exp (bf16 x bf16 -> 4x DVE rate)
- projections stream weights in L-chunks of 384 with k-outer/o-inner loops,
  FFN is fused (w1 block -> relu -> w2 block) to bound PSUM/SBUF residency
"""
import os
import numpy as np
import ml_dtypes

import concourse.bass as bass
from concourse import bacc
import concourse.mybir as mybir
import concourse.tile as tile
from concourse.bass_utils import run_bass_kernel_spmd

F32 = mybir.dt.float32
BF16 = mybir.dt.bfloat16
AF = mybir.ActivationFunctionType
OP = mybir.AluOpType

B, Tt, Ta, L, D, H, DH, DFF, NL = 4, 128, 640, 768, 1024, 16, 64, 4096, 6
VT, VA = 256, 1026
NLAYERS = int(os.environ.get("KERNEL_NL", str(NL)))
P = 128
NT = L // P          # 6 sequence tiles
DK = D // P          # 8 feature tiles
QO = 4               # local q/k/ctx tiles (512 dims)
OO = 8               # output tiles of D
HLOC = 8             # local heads
HEADO = 5            # head out-tiles (640-row padded vocab slab)
CHN = (0, 384, 768)  # L chunks for streamed projections
# SA (tk allowed) per 256-wide q chunk c (q tiles 2c, 2c+1)
SA_SET = {0: (0, 1), 1: (0, 1, 2, 3), 2: (0, 1, 2, 3, 4, 5)}


def _build_nc():
    nc = bacc.Bacc(None)

    x0_d = nc.declare_dram_parameter("x0", [D, L], F32, isOutput=False)
    mem_d = nc.declare_dram_parameter("mem0", [D, L], BF16, isOutput=False)
    mask_d = nc.declare_dram_parameter("maskT", [P, NT * L], BF16, isOutput=False)
    wqk_sa = nc.declare_dram_parameter("wqk_sa", [NLAYERS, D, 1024], BF16, isOutput=False)
    wv_sa = nc.declare_dram_parameter("wv_sa", [NLAYERS, D, 512], BF16, isOutput=False)
    wo_sa = nc.declare_dram_parameter("wo_sa", [NLAYERS, 512, D], BF16, isOutput=False)
    wq_ca = nc.declare_dram_parameter("wq_ca", [NLAYERS, D, 512], BF16, isOutput=False)
    wk_ca = nc.declare_dram_parameter("wk_ca", [NLAYERS, D, 512], BF16, isOutput=False)
    wv_ca = nc.declare_dram_parameter("wv_ca", [NLAYERS, D, 512], BF16, isOutput=False)
    wo_ca = nc.declare_dram_parameter("wo_ca", [NLAYERS, 512, D], BF16, isOutput=False)
    w1_d = nc.declare_dram_parameter("w1T", [NLAYERS, D, 2048], BF16, isOutput=False)
    w2_d = nc.declare_dram_parameter("w2T", [NLAYERS, 2048, D], BF16, isOutput=False)
    head_d = nc.declare_dram_parameter("headT", [D, HEADO * P], BF16, isOutput=False)
    logits = nc.declare_dram_parameter("logits", [HEADO * P, L], BF16, isOutput=True)

    DBG = bool(int(os.environ.get("KERNEL_DEBUG", "0")))
    dbg = {}
    if DBG:
        for nm, shp, dt in [("dh1", [D, L], BF16), ("dqk", [1024, L], BF16),
                            ("dvsa", [P, NT * HLOC * 66], BF16),
                            ("dctx", [512, L], BF16), ("dx1", [D, L], F32),
                            ("dcak", [512, L], BF16), ("dctx2", [512, L], BF16),
                            ("dx2", [D, L], F32), ("dx3", [D, L], F32)]:
            dbg[nm] = nc.declare_dram_parameter(nm, shp, dt, isOutput=True)

    cc_in = nc.dram_tensor("cc_in", [D, L], BF16)
    cc_out = nc.dram_tensor("cc_out", [D, L], BF16)
    GROUPS = [[0, 1], [2, 3], [4, 5], [6, 7]]

    from contextlib import ExitStack
    with tile.TileContext(nc) as tc, ExitStack() as S:
        state = S.enter_context(tc.tile_pool(name="state", bufs=1))
        wpool = S.enter_context(tc.tile_pool(name="wpool", bufs=20))
        evb = S.enter_context(tc.tile_pool(name="evb", bufs=3))
        prb = S.enter_context(tc.tile_pool(name="prb", bufs=4))
        xsqp = S.enter_context(tc.tile_pool(name="xsqp", bufs=2))
        tmpp = S.enter_context(tc.tile_pool(name="tmpp", bufs=2))
        bcp = S.enter_context(tc.tile_pool(name="bcp", bufs=2))
        bcb = S.enter_context(tc.tile_pool(name="bcb", bufs=2))
        h1p = S.enter_context(tc.tile_pool(name="h1p", bufs=2))
        invp = S.enter_context(tc.tile_pool(name="invp", bufs=2))

        xf = state.tile([P, DK, L], F32)
        mem = state.tile([P, DK, L], BF16)
        hT = state.tile([P, DK, L], BF16)
        big = state.tile([P, OO, L], BF16)       # SA q(0:4)+k(4:8) / CA q(0:4)
        cak = state.tile([P, QO, L], BF16)       # CA kT
        ctxT = state.tile([P, QO, L], BF16)
        vsa = state.tile([P, NT, HLOC, 66], BF16)
        vca = state.tile([P, NT, HLOC, 66], BF16)
        mk = state.tile([P, NT, L], BF16)
        wvr_sa = state.tile([P, DK, 512], BF16)  # resident v weights
        wvr_ca = state.tile([P, DK, 512], BF16)
        onesb = state.tile([P, 1], BF16)
        epst = state.tile([1, 1], F32)
        mu_s = state.tile([1, L], F32)
        var_s = state.tile([1, L], F32)
        sd_s = state.tile([1, L], F32)
        sdb_s = state.tile([1, L], BF16)

        nc.vector.memset(onesb, 1.0)
        nc.vector.memset(epst, 1e-5)
        nc.vector.memset(vsa[:, :, :, 64:66], 0.0)
        nc.vector.memset(vca[:, :, :, 64:66], 0.0)
        nc.vector.memset(vsa[:, :, :, 64:65], 1.0)
        nc.vector.memset(vca[:, :, :, 64:65], 1.0)
        for k in range(DK):
            nc.sync.dma_start(out=xf[:, k, :], in_=x0_d[k * P:(k + 1) * P, :])
            nc.sync.dma_start(out=mem[:, k, :], in_=mem_d[k * P:(k + 1) * P, :])
        for t in range(NT):
            nc.sync.dma_start(out=mk[:, t, :], in_=mask_d[:, t * L:(t + 1) * L])

        def dump(nm, tile_ap, n, w=L):
            if not DBG:
                return
            d = dbg[nm]
            for o in range(n):
                nc.sync.dma_start(out=d[o * P:(o + 1) * P, :], in_=tile_ap[:, o, :])

        # ---------------- helpers ----------------
        def layernorm():
            """LN over partition dim of xf -> hT (no affine; w=1, b=0)."""
            with tc.tile_pool(name="ln_ps", bufs=1, space="PSUM") as lps:
                s12 = lps.tile([33, L], F32)
                for k in range(DK):
                    xsq = xsqp.tile([P, 2, L], BF16, tag="xsq", name="xsq")
                    nc.scalar.copy(xsq[:, 0, :], xf[:, k, :])
                    nc.scalar.activation(xsq[:, 1, :], xf[:, k, :], AF.Square)
                    st, sp = (k == 0), (k == DK - 1)
                    for c0, c1 in ((0, 512), (512, L)):
                        nc.tensor.matmul(s12[0:1, c0:c1], onesb, xsq[:, 0, c0:c1],
                                         start=st, stop=sp)
                        nc.tensor.matmul(s12[32:33, c0:c1], onesb, xsq[:, 1, c0:c1],
                                         start=st, stop=sp)
                nc.vector.tensor_scalar_mul(mu_s, s12[0:1, :], 1.0 / D)
                nc.vector.tensor_mul(out=var_s, in0=mu_s, in1=mu_s)
                nc.vector.scalar_tensor_tensor(
                    out=var_s, in0=s12[32:33, :], scalar=1.0 / D,
                    in1=var_s, op0=OP.mult, op1=OP.subtract)
            nc.scalar.activation(sd_s, var_s, AF.Sqrt, bias=epst[0:1, 0:1])
            nc.vector.reciprocal(out=sd_s, in_=sd_s)
            nc.scalar.copy(sdb_s, sd_s)
            mub = bcp.tile([P, L], F32, tag="mub", name="mub")
            nc.gpsimd.partition_broadcast(mub, mu_s[0:1, :])
            rb = bcb.tile([P, L], BF16, tag="rb", name="rb")
            nc.gpsimd.partition_broadcast(rb, sdb_s[0:1, :])
            for k in range(DK):
                t = tmpp.tile([P, L], BF16, tag="lt", name="lt")
                nc.vector.tensor_tensor(out=t, in0=xf[:, k, :], in1=mub,
                                        op=OP.subtract)
                nc.vector.tensor_mul(out=hT[:, k, :], in0=t, in1=rb)

        def proj(w_ap, n_o, n_k, wcols, rhs_fn, out_fn):
            """Streamed projection: out[o] = sum_k W^T[k]-tile @ rhs(k).

            k-outer / o-inner with per-L-chunk accumulators so only one
            weight k-tile is live at a time. w_ap: [Din, wcols] dram slice,
            rhs_fn(k) -> bf16 AP [128, L]; out_fn(o, acc, c0, c1).
            """
            with tc.tile_pool(name="pj_ps", bufs=1, space="PSUM") as pps:
                for ci in range(2):
                    c0, c1 = CHN[ci], CHN[ci + 1]
                    accs = [pps.tile([P, 384], F32, tag=f"pacc{o}",
                                     name=f"pacc{o}") for o in range(n_o)]
                    for k in range(n_k):
                        wt = wpool.tile([P, 1024], BF16, tag="w", name="wt")
                        nc.sync.dma_start(out=wt[:, 0:wcols],
                                          in_=w_ap[k * P:(k + 1) * P, :])
                        rhs = rhs_fn(k)
                        for o in range(n_o):
                            nc.tensor.matmul(accs[o], wt[:, o * P:(o + 1) * P],
                                             rhs[:, c0:c1],
                                             start=(k == 0), stop=(k == n_k - 1))
                    for o in range(n_o):
                        out_fn(o, accs[o], c0, c1)

        def vproj(wvr, src, vdst):
            """Token-major V: vdst[:, t, h, 0:64] = (src^T W_v^T)[t-tile]."""
            with tc.tile_pool(name="v_ps", bufs=2, space="PSUM") as vps:
                for t in range(NT):
                    vacc = vps.tile([P, 512], F32, tag="vacc", name="vacc")
                    for k in range(DK):
                        nc.tensor.matmul(vacc, src[:, k, t * P:(t + 1) * P],
                                         wvr[:, k, :],
                                         start=(k == 0), stop=(k == DK - 1))
                    nc.vector.tensor_copy(out=vdst[:, t, :, 0:64], in_=vacc)

        def attention(masked, vtok, kt_fn):
            """softmax((big q)^T k / 8 + mask) @ v -> ctxT (denom via ones col)."""
            with tc.tile_pool(name="at_sps", bufs=2, space="PSUM") as sps, \
                 tc.tile_pool(name="at_cps", bufs=3, space="PSUM") as cps:
                for h in range(HLOC):
                    j, hb = h // 2, 64 * (h % 2)
                    if masked:
                        chunks = [(256 * c, 256, SA_SET[c]) for c in range(3)]
                    else:
                        chunks = [(0, 512, tuple(range(NT))),
                                  (512, 256, tuple(range(NT)))]
                    for q0, qw, tks in chunks:
                        ctx = cps.tile([P, 512], F32, tag="ctx", name="ctx")
                        for ti, tk in enumerate(tks):
                            sc = sps.tile([P, 512], F32, tag="sc", name="sc")
                            nc.tensor.matmul(
                                sc[:, 0:qw],
                                kt_fn(j)[hb:hb + 64, tk * P:(tk + 1) * P],
                                big[hb:hb + 64, j, q0:q0 + qw],
                                start=True, stop=True)
                            pr = prb.tile([P, 512], BF16, tag="pr", name="pr")
                            nc.scalar.activation(pr[:, 0:qw], sc[:, 0:qw],
                                                 AF.Exp, scale=0.125)
                            if masked:
                                nc.vector.tensor_mul(
                                    out=pr[:, 0:qw], in0=pr[:, 0:qw],
                                    in1=mk[:, tk, q0:q0 + qw])
                            nc.tensor.matmul(ctx[0:65, 0:qw],
                                             vtok[:, tk, h, 0:65], pr[:, 0:qw],
                                             start=(ti == 0),
                                             stop=(ti == len(tks) - 1))
                        inv1 = invp.tile([1, 512], F32, tag="inv", name="inv1")
                        nc.vector.reciprocal(out=inv1[:, 0:qw],
                                             in_=ctx[64:65, 0:qw])
                        invb = bcp.tile([P, 512], F32, tag="invb", name="invb")
                        nc.gpsimd.partition_broadcast(invb[:, 0:qw],
                                                      inv1[0:1, 0:qw])
                        nc.vector.tensor_mul(
                            out=ctxT[hb:hb + 64, j, q0:q0 + qw],
                            in0=ctx[0:64, 0:qw], in1=invb[0:64, 0:qw])

        def out_evac(o, acc, c0, c1):
            ev = evb.tile([P, 384], BF16, tag="ev", name="ev")
            nc.vector.tensor_copy(out=ev[:, 0:c1 - c0], in_=acc[:, 0:c1 - c0])
            nc.sync.dma_start(out=cc_in[o * P:(o + 1) * P, c0:c1],
                              in_=ev[:, 0:c1 - c0])

        def ar_issue():
            nc.gpsimd.collective_compute(
                "AllReduce", OP.add, replica_groups=GROUPS,
                ins=[cc_in[:, :]], outs=[cc_out[:, :]])

        def ar_accum():
            for o in range(DK):
                rr = evb.tile([P, L], BF16, tag="rr", name="rr")
                nc.sync.dma_start(out=rr, in_=cc_out[o * P:(o + 1) * P, :])
                nc.vector.tensor_tensor(out=xf[:, o, :], in0=xf[:, o, :],
                                        in1=rr, op=OP.add)

        def qk_evac(o, acc, c0, c1):
            nc.vector.tensor_copy(out=big[:, o, c0:c1], in_=acc[:, 0:c1 - c0])

        def cak_evac(o, acc, c0, c1):
            nc.vector.tensor_copy(out=cak[:, o, c0:c1], in_=acc[:, 0:c1 - c0])

        def ffn():
            """Fused w1 -> relu -> w2, DFF in 2 blocks of 1024, L in 2 chunks."""
            with tc.tile_pool(name="ffn_ps", bufs=1, space="PSUM") as fps, \
                 tc.tile_pool(name="a1_ps", bufs=2, space="PSUM") as aps:
                for ci in range(2):
                    c0, c1 = CHN[ci], CHN[ci + 1]
                    accs = [fps.tile([P, 384], F32, tag=f"facc{o}",
                                     name=f"facc{o}") for o in range(OO)]
                    for jb in range(2):
                        w1t = []
                        for k in range(DK):
                            w = wpool.tile([P, 1024], BF16, tag="w", name="w1t")
                            nc.sync.dma_start(
                                out=w,
                                in_=w1_d[l, k * P:(k + 1) * P,
                                         jb * 1024:(jb + 1) * 1024])
                            w1t.append(w)
                        h1b = h1p.tile([P, DK, 384], BF16, tag="h1b", name="h1b")
                        for oo in range(DK):
                            a1 = aps.tile([P, 384], F32, tag="a1", name="a1")
                            for k in range(DK):
                                nc.tensor.matmul(
                                    a1, w1t[k][:, oo * P:(oo + 1) * P],
                                    hT[:, k, c0:c1],
                                    start=(k == 0), stop=(k == DK - 1))
                            nc.scalar.activation(h1b[:, oo, :], a1, AF.Relu)
                        for k2 in range(DK):
                            w2t = wpool.tile([P, 1024], BF16, tag="w", name="w2t")
                            nc.sync.dma_start(
                                out=w2t,
                                in_=w2_d[l, jb * 1024 + k2 * P:
                                         jb * 1024 + (k2 + 1) * P, :])
                            for o in range(OO):
                                nc.tensor.matmul(
                                    accs[o], w2t[:, o * P:(o + 1) * P],
                                    h1b[:, k2, :],
                                    start=(jb == 0 and k2 == 0),
                                    stop=(jb == 1 and k2 == DK - 1))
                    for o in range(OO):
                        out_evac(o, accs[o], c0, c1)

        # ---------------- resident V weights for layer 0 ----------------
        def load_wv(wvr, w_ap):
            for k in range(DK):
                nc.sync.dma_start(out=wvr[:, k, :], in_=w_ap[k * P:(k + 1) * P, :])

        # ---------------- layers ----------------
        for l in range(NLAYERS):
            load_wv(wvr_sa, wv_sa[l])
            # ---- self-attention ----
            layernorm()
            if l == 0:
                dump("dh1", hT, DK)
            vproj(wvr_sa, hT, vsa)
            proj(wqk_sa[l], OO, DK, 1024, lambda k: hT[:, k, :], qk_evac)
            if l == 0:
                dump("dqk", big, OO)
                if DBG:
                    nc.sync.dma_start(out=dbg["dvsa"][:, :],
                                      in_=vsa[:, :, :, :].reshape([P, NT * HLOC * 66]))
            attention(True, vsa, lambda j: big[:, 4 + j, :])
            if l == 0:
                dump("dctx", ctxT, QO)
            proj(wo_sa[l], OO, QO, 1024, lambda k: ctxT[:, k, :], out_evac)
            ar_issue()
            # AR1 shadow: CA k/v from mem (x-independent)
            load_wv(wvr_ca, wv_ca[l])
            proj(wk_ca[l], QO, DK, 512, lambda k: mem[:, k, :], cak_evac)
            vproj(wvr_ca, mem, vca)
            ar_accum()
            if l == 0:
                dump("dx1", xf, DK)

            # ---- cross-attention ----
            layernorm()
            proj(wq_ca[l], QO, DK, 512, lambda k: hT[:, k, :], qk_evac)
            if l == 0:
                dump("dcak", cak, QO)
            attention(False, vca, lambda j: cak[:, j, :])
            if l == 0:
                dump("dctx2", ctxT, QO)
            proj(wo_ca[l], OO, QO, 1024, lambda k: ctxT[:, k, :], out_evac)
            ar_issue()
            ar_accum()
            if l == 0:
                dump("dx2", xf, DK)

            # ---- FFN ----
            layernorm()
            ffn()
            ar_issue()
            ar_accum()
            if l == 0:
                dump("dx3", xf, DK)

        # ---------------- head ----------------
        for k in range(DK):
            nc.scalar.copy(hT[:, k, :], xf[:, k, :])

        def head_evac(o, acc, c0, c1):
            ev = evb.tile([P, 384], BF16, tag="ev", name="ev")
            nc.scalar.copy(ev[:, 0:c1 - c0], acc[:, 0:c1 - c0])
            nc.sync.dma_start(out=logits[o * P:(o + 1) * P, c0:c1],
                              in_=ev[:, 0:c1 - c0])

        proj(head_d, HEADO, DK, HEADO * P, lambda k: hT[:, k, :], head_evac)

    nc.finalize()
    return nc


# ---------------------------------------------------------------------------
# host side
# ---------------------------------------------------------------------------

def _pe_table(length, d):
    pos = np.arange(length, dtype=np.float32)[:, None]
    div = np.exp(np.arange(0, d, 2, dtype=np.float32) * (-np.log(10000.0) / d))
    ang = pos * div
    out = np.zeros((length, d), np.float32)
    out[:, 0::2] = np.sin(ang)
    out[:, 1::2] = np.cos(ang)
    return out


BF = ml_dtypes.bfloat16


def _bf(a):
    return np.ascontiguousarray(np.asarray(a, np.float32).astype(BF))


_NC_CACHE = {}
LAST_RESULT = {}


def kernel(**inputs):
    f32 = lambda a: np.asarray(a, dtype=np.float32)
    text = np.asarray(inputs["text"]).astype(np.int64)
    audio = np.asarray(inputs["audio"]).astype(np.int64)
    tl = np.asarray(inputs["text_len_batch"]).astype(np.int64)
    al = np.asarray(inputs["audio_len_batch"]).astype(np.int64)
    text_table = f32(inputs["text_table"])
    audio_table = f32(inputs["audio_table"])
    sa_in_w = f32(inputs["sa_in_w"])
    sa_out_w = f32(inputs["sa_out_w"])
    ca_in_w = f32(inputs["ca_in_w"])
    ca_out_w = f32(inputs["ca_out_w"])
    ffn_w1 = f32(inputs["ffn_w1"])
    ffn_w2 = f32(inputs["ffn_w2"])

    pe_t = _pe_table(Tt, D)
    pe_a = _pe_table(Ta, D)

    # per-TP-rank weight shards (shared by the 4 cores with the same rank)
    shard = []
    for r in range(2):
        sl = slice(512 * r, 512 * (r + 1))
        fl = slice(2048 * r, 2048 * (r + 1))
        wq, wk, wv = (sa_in_w[:NLAYERS, i * D:(i + 1) * D, :][:, sl]
                      for i in range(3))
        cq, ck, cv = (ca_in_w[:NLAYERS, i * D:(i + 1) * D, :][:, sl]
                      for i in range(3))
        hw = np.zeros((HEADO * P, D), np.float32)
        hw[0:513] = audio_table[513 * r:513 * (r + 1)]
        shard.append({
            "wqk_sa": _bf(np.concatenate([wq, wk], axis=1).transpose(0, 2, 1)),
            "wv_sa": _bf(wv.transpose(0, 2, 1)),
            "wo_sa": _bf(sa_out_w[:NLAYERS, :, sl].transpose(0, 2, 1)),
            "wq_ca": _bf(cq.transpose(0, 2, 1)),
            "wk_ca": _bf(ck.transpose(0, 2, 1)),
            "wv_ca": _bf(cv.transpose(0, 2, 1)),
            "wo_ca": _bf(ca_out_w[:NLAYERS, :, sl].transpose(0, 2, 1)),
            "w1T": _bf(ffn_w1[:NLAYERS, fl, :].transpose(0, 2, 1)),
            "w2T": _bf(ffn_w2[:NLAYERS, :, fl].transpose(0, 2, 1)),
            "headT": _bf(hw.T),
        })

    in_maps = []
    for c in range(8):
        p, r = c // 2, c % 2
        tlb, alb = int(tl[p]), int(al[p])
        il = tlb + alb

        emb = np.zeros((L, D), np.float32)
        emb[:tlb] = text_table[text[p, :tlb]] + pe_t[:tlb]
        emb[tlb:il] = audio_table[audio[p, :alb]] + pe_a[:alb]
        x0 = np.ascontiguousarray(emb.T)

        kk = np.arange(L)
        allowed = ((kk[None, :] < tlb)
                   | ((kk[None, :] <= kk[:, None]) & (kk[:, None] < il)))
        # maskT[k_part, tk*L + q] = allowed(q, tk*128 + k_part)
        mt = allowed.T.reshape(NT, P, L).transpose(1, 0, 2).reshape(P, NT * L)

        m = {"x0": x0, "mem0": _bf(x0), "maskT": _bf(mt)}
        m.update(shard[r])
        in_maps.append(m)

    key = (NLAYERS, os.environ.get("KERNEL_DEBUG", "0"))
    if _NC_CACHE.get("key") != key:
        _NC_CACHE["nc"] = _build_nc()
        _NC_CACHE["key"] = key
    nc = _NC_CACHE["nc"]
    trace = bool(int(os.environ.get("KERNEL_TRACE", "0")))
    r = run_bass_kernel_spmd(nc, in_maps, core_ids=list(range(8)), trace=trace)
    LAST_RESULT["r"] = r
    res = r.results

    out = np.empty((B, L, VA), np.float32)
    for p in range(B):
        ev = np.asarray(res[2 * p]["logits"], dtype=np.float32)
        od = np.asarray(res[2 * p + 1]["logits"], dtype=np.float32)
        out[p] = np.concatenate([ev[0:513], od[0:513]], axis=0).T
    return out


# revision 9
# speedup vs baseline: 1.5689x; 1.5689x over previous
"""Trainium2 Bass kernel for nn_AutoRegressive (dense transformer decoder), v2.

Model: B=4 packed text+audio sequences, L=768, D=1024, 16 heads, DFF=4096,
6 norm-first decoder layers (self-attn w/ prefix-LM mask, cross-attn to the
packed embedding, FFN), weight-tied audio head. fp32 inputs/outputs.

Sharding: DP4 x TP2 over 8 cores. Core pair (2i, 2i+1) owns batch item i;
within a pair the 16 heads split 8+8 and DFF splits 2048+2048. Three pair-
AllReduces per layer; CA K/V projections are scheduled into AR shadows.

v2 design vs baseline:
- all weights pre-transposed + bf16 on host (no PE transposes, no ACT copies)
- activations bf16 (residual stream f32), matmuls bf16 @ 1 cyc/row
- embedding + positional encoding gathered/packed on host -> x0 DMA
- V projected token-major directly (stationary = activation tile), so AV
  stationary needs no on-device transpose; ones column gives softmax denom
- SA score tiles that are fully masked are skipped (12 of 18 (tk,chunk))
- mask applied multiplicatively AFTER exp (bf16 x bf16 -> 4x DVE rate)
- projections stream weights in L-chunks of 384 with k-outer/o-inner loops,
  FFN is fused (w1 block -> relu -> w2 block) to bound PSUM/SBUF residency
"""
import os
import numpy as np
import ml_dtypes

import concourse.bass as bass
from concourse import bacc
import concourse.mybir as mybir
import concourse.tile as tile
from concourse.bass_utils import run_bass_kernel_spmd

F32 = mybir.dt.float32
BF16 = mybir.dt.bfloat16
AF = mybir.ActivationFunctionType
OP = mybir.AluOpType

B, Tt, Ta, L, D, H, DH, DFF, NL = 4, 128, 640, 768, 1024, 16, 64, 4096, 6
VT, VA = 256, 1026
NLAYERS = int(os.environ.get("KERNEL_NL", str(NL)))
P = 128
NT = L // P          # 6 sequence tiles
DK = D // P          # 8 feature tiles
QO = 4               # local q/k/ctx tiles (512 dims)
OO = 8               # output tiles of D
HLOC = 8             # local heads
HEADO = 5            # head out-tiles (640-row padded vocab slab)
CHN = (0, 384, 768)  # L chunks for streamed projections
# SA (tk allowed) per 256-wide q chunk c (q tiles 2c, 2c+1)
SA_SET = {0: (0, 1), 1: (0, 1, 2, 3), 2: (0, 1, 2, 3, 4, 5)}


def _build_nc():
    nc = bacc.Bacc(None)

    x0_d = nc.declare_dram_parameter("x0", [D, L], F32, isOutput=False)
    mem_d = nc.declare_dram_parameter("mem0", [D, L], BF16, isOutput=False)
    mask_d = nc.declare_dram_parameter("maskT", [P, NT * L], BF16, isOutput=False)
    wqk_sa = nc.declare_dram_parameter("wqk_sa", [NLAYERS, D, 1024], BF16, isOutput=False)
    wv_sa = nc.declare_dram_parameter("wv_sa", [NLAYERS, D, 512], BF16, isOutput=False)
    wo_sa = nc.declare_dram_parameter("wo_sa", [NLAYERS, 512, D], BF16, isOutput=False)
    wq_ca = nc.declare_dram_parameter("wq_ca", [NLAYERS, D, 512], BF16, isOutput=False)
    wk_ca = nc.declare_dram_parameter("wk_ca", [NLAYERS, D, 512], BF16, isOutput=False)
    wv_ca = nc.declare_dram_parameter("wv_ca", [NLAYERS, D, 512], BF16, isOutput=False)
    wo_ca = nc.declare_dram_parameter("wo_ca", [NLAYERS, 512, D], BF16, isOutput=False)
    w1_d = nc.declare_dram_parameter("w1T", [NLAYERS, D, 2048], BF16, isOutput=False)
    w2_d = nc.declare_dram_parameter("w2T", [NLAYERS, 2048, D], BF16, isOutput=False)
    head_d = nc.declare_dram_parameter("headT", [D, HEADO * P], BF16, isOutput=False)
    logits = nc.declare_dram_parameter("logits", [HEADO * P, L], BF16, isOutput=True)

    DBG = bool(int(os.environ.get("KERNEL_DEBUG", "0")))
    dbg = {}
    if DBG:
        for nm, shp, dt in [("dh1", [D, L], BF16), ("dqk", [1024, L], BF16),
                            ("dctx", [512, L], BF16), ("dx1", [D, L], F32),
                            ("dcak", [512, L], BF16), ("dctx2", [512, L], BF16),
                            ("dx2", [D, L], F32), ("dx3", [D, L], F32)]:
            dbg[nm] = nc.declare_dram_parameter(nm, shp, dt, isOutput=True)

    cc_in = nc.dram_tensor("cc_in", [D, L], BF16)
    cc_out = nc.dram_tensor("cc_out", [D, L], BF16)
    GROUPS = [[0, 1], [2, 3], [4, 5], [6, 7]]

    from contextlib import ExitStack
    with tile.TileContext(nc) as tc, ExitStack() as S:
        state = S.enter_context(tc.tile_pool(name="state", bufs=1))
        wpool = S.enter_context(tc.tile_pool(name="wpool", bufs=18))
        evb = S.enter_context(tc.tile_pool(name="evb", bufs=3))
        prb = S.enter_context(tc.tile_pool(name="prb", bufs=4))
        xsqp = S.enter_context(tc.tile_pool(name="xsqp", bufs=2))
        tmpp = S.enter_context(tc.tile_pool(name="tmpp", bufs=2))
        bcp = S.enter_context(tc.tile_pool(name="bcp", bufs=2))
        bcb = S.enter_context(tc.tile_pool(name="bcb", bufs=2))
        h1p = S.enter_context(tc.tile_pool(name="h1p", bufs=2))
        invp = S.enter_context(tc.tile_pool(name="invp", bufs=2))

        xf = state.tile([P, DK, L], F32)
        mem = state.tile([P, DK, L], BF16)
        hT = state.tile([P, DK, L], BF16)
        big = state.tile([P, OO, L], BF16)       # SA q(0:4)+k(4:8) / CA q(0:4)
        cak = state.tile([P, QO, L], BF16)       # CA kT
        ctxT = state.tile([P, QO, L], BF16)
        vsa = state.tile([P, NT, HLOC, 66], BF16)
        vca = state.tile([P, NT, HLOC, 66], BF16)
        mk = state.tile([P, NT, L], BF16)
        wvr_sa = state.tile([P, DK, 512], BF16)  # resident v weights
        wvr_ca = state.tile([P, DK, 512], BF16)
        onesb = state.tile([P, 1], BF16)
        epst = state.tile([1, 1], F32)
        mu_s = state.tile([1, L], F32)
        var_s = state.tile([1, L], F32)
        sd_s = state.tile([1, L], F32)
        sdb_s = state.tile([1, L], BF16)

        nc.vector.memset(onesb, 1.0)
        nc.vector.memset(epst, 1e-5)
        nc.vector.memset(vsa[:, :, :, 64:65], 1.0)
        nc.vector.memset(vca[:, :, :, 64:65], 1.0)
        for k in range(DK):
            nc.sync.dma_start(out=xf[:, k, :], in_=x0_d[k * P:(k + 1) * P, :])
            nc.sync.dma_start(out=mem[:, k, :], in_=mem_d[k * P:(k + 1) * P, :])
        for t in range(NT):
            nc.sync.dma_start(out=mk[:, t, :], in_=mask_d[:, t * L:(t + 1) * L])

        def dump(nm, tile_ap, n, w=L):
            if not DBG:
                return
            d = dbg[nm]
            for o in range(n):
                nc.sync.dma_start(out=d[o * P:(o + 1) * P, :], in_=tile_ap[:, o, :])

        # ---------------- helpers ----------------
        def layernorm():
            """LN over partition dim of xf -> hT (no affine; w=1, b=0)."""
            with tc.tile_pool(name="ln_ps", bufs=1, space="PSUM") as lps:
                s12 = lps.tile([33, L], F32)
                for k in range(DK):
                    xsq = xsqp.tile([P, 2, L], BF16, tag="xsq", name="xsq")
                    nc.scalar.copy(xsq[:, 0, :], xf[:, k, :])
                    nc.scalar.activation(xsq[:, 1, :], xf[:, k, :], AF.Square)
                    st, sp = (k == 0), (k == DK - 1)
                    for c0, c1 in ((0, 512), (512, L)):
                        nc.tensor.matmul(s12[0:1, c0:c1], onesb, xsq[:, 0, c0:c1],
                                         start=st, stop=sp)
                        nc.tensor.matmul(s12[32:33, c0:c1], onesb, xsq[:, 1, c0:c1],
                                         start=st, stop=sp)
                nc.vector.tensor_scalar_mul(mu_s, s12[0:1, :], 1.0 / D)
                nc.vector.tensor_mul(out=var_s, in0=mu_s, in1=mu_s)
                nc.vector.scalar_tensor_tensor(
                    out=var_s, in0=s12[32:33, :], scalar=1.0 / D,
                    in1=var_s, op0=OP.mult, op1=OP.subtract)
            nc.scalar.activation(sd_s, var_s, AF.Sqrt, bias=epst[0:1, 0:1])
            nc.vector.reciprocal(out=sd_s, in_=sd_s)
            nc.scalar.copy(sdb_s, sd_s)
            mub = bcp.tile([P, L], F32, tag="mub", name="mub")
            nc.gpsimd.partition_broadcast(mub, mu_s[0:1, :])
            rb = bcb.tile([P, L], BF16, tag="rb", name="rb")
            nc.gpsimd.partition_broadcast(rb, sdb_s[0:1, :])
            for k in range(DK):
                t = tmpp.tile([P, L], BF16, tag="lt", name="lt")
                nc.vector.tensor_tensor(out=t, in0=xf[:, k, :], in1=mub,
                                        op=OP.subtract)
                nc.vector.tensor_mul(out=hT[:, k, :], in0=t, in1=rb)

        def proj(w_ap, n_o, n_k, wcols, rhs_fn, out_fn):
            """Streamed projection: out[o] = sum_k W^T[k]-tile @ rhs(k).

            k-outer / o-inner with per-L-chunk accumulators so only one
            weight k-tile is live at a time. w_ap: [Din, wcols] dram slice,
            rhs_fn(k) -> bf16 AP [128, L]; out_fn(o, acc, c0, c1).
            """
            with tc.tile_pool(name="pj_ps", bufs=1, space="PSUM") as pps:
                for ci in range(2):
                    c0, c1 = CHN[ci], CHN[ci + 1]
                    accs = [pps.tile([P, 384], F32, tag=f"pacc{o}",
                                     name=f"pacc{o}") for o in range(n_o)]
                    for k in range(n_k):
                        wt = wpool.tile([P, 1024], BF16, tag="w", name="wt")
                        nc.sync.dma_start(out=wt[:, 0:wcols],
                                          in_=w_ap[k * P:(k + 1) * P, :])
                        rhs = rhs_fn(k)
                        for o in range(n_o):
                            nc.tensor.matmul(accs[o], wt[:, o * P:(o + 1) * P],
                                             rhs[:, c0:c1],
                                             start=(k == 0), stop=(k == n_k - 1))
                    for o in range(n_o):
                        out_fn(o, accs[o], c0, c1)

        def vproj(wvr, src, vdst):
            """Token-major V: vdst[:, t, h, 0:64] = (src^T W_v^T)[t-tile]."""
            with tc.tile_pool(name="v_ps", bufs=2, space="PSUM") as vps:
                for t in range(NT):
                    vacc = vps.tile([P, DK, 64], F32, tag="vacc", name="vacc")
                    for k in range(DK):
                        nc.tensor.matmul(vacc[:, :, :],
                                         src[:, k, t * P:(t + 1) * P],
                                         wvr[:, k, :],
                                         start=(k == 0), stop=(k == DK - 1))
                    nc.vector.tensor_copy(out=vdst[:, t, :, 0:64],
                                          in_=vacc[:, :, :])

        def attention(masked, vtok, kt_fn):
            """softmax((big q)^T k / 8 + mask) @ v -> ctxT (denom via ones col)."""
            with tc.tile_pool(name="at_sps", bufs=2, space="PSUM") as sps, \
                 tc.tile_pool(name="at_cps", bufs=3, space="PSUM") as cps:
                for h in range(HLOC):
                    j, hb = h // 2, 64 * (h % 2)
                    if masked:
                        chunks = [(256 * c, 256, SA_SET[c]) for c in range(3)]
                    else:
                        chunks = [(0, 512, tuple(range(NT))),
                                  (512, 256, tuple(range(NT)))]
                    for q0, qw, tks in chunks:
                        ctx = cps.tile([P, 512], F32, tag="ctx", name="ctx")
                        for ti, tk in enumerate(tks):
                            sc = sps.tile([P, 512], F32, tag="sc", name="sc")
                            nc.tensor.matmul(
                                sc[:, 0:qw],
                                kt_fn(j)[hb:hb + 64, tk * P:(tk + 1) * P],
                                big[hb:hb + 64, j, q0:q0 + qw],
                                start=True, stop=True)
                            pr = prb.tile([P, 512], BF16, tag="pr", name="pr")
                            nc.scalar.activation(pr[:, 0:qw], sc[:, 0:qw],
                                                 AF.Exp, scale=0.125)
                            if masked:
                                nc.vector.tensor_mul(
                                    out=pr[:, 0:qw], in0=pr[:, 0:qw],
                                    in1=mk[:, tk, q0:q0 + qw])
                            nc.tensor.matmul(ctx[0:65, 0:qw],
                                             vtok[:, tk, h, 0:65], pr[:, 0:qw],
                                             start=(ti == 0),
                                             stop=(ti == len(tks) - 1))
                        inv1 = invp.tile([1, 512], F32, tag="inv", name="inv1")
                        nc.vector.reciprocal(out=inv1[:, 0:qw],
                                             in_=ctx[64:65, 0:qw])
                        invb = bcp.tile([P, 512], F32, tag="invb", name="invb")
                        nc.gpsimd.partition_broadcast(invb[:, 0:qw],
                                                      inv1[0:1, 0:qw])
                        nc.vector.tensor_mul(
                            out=ctxT[hb:hb + 64, j, q0:q0 + qw],
                            in0=ctx[0:64, 0:qw], in1=invb[0:64, 0:qw])

        def out_evac(o, acc, c0, c1):
            ev = evb.tile([P, 384], BF16, tag="ev", name="ev")
            nc.vector.tensor_copy(out=ev[:, 0:c1 - c0], in_=acc[:, 0:c1 - c0])
            nc.sync.dma_start(out=cc_in[o * P:(o + 1) * P, c0:c1],
                              in_=ev[:, 0:c1 - c0])

        def ar_issue():
            nc.gpsimd.collective_compute(
                "AllReduce", OP.add, replica_groups=GROUPS,
                ins=[cc_in[:, :]], outs=[cc_out[:, :]])

        def ar_accum():
            for o in range(DK):
                rr = evb.tile([P, L], BF16, tag="rr", name="rr")
                nc.sync.dma_start(out=rr, in_=cc_out[o * P:(o + 1) * P, :])
                nc.vector.tensor_tensor(out=xf[:, o, :], in0=xf[:, o, :],
                                        in1=rr, op=OP.add)

        def qk_evac(o, acc, c0, c1):
            nc.vector.tensor_copy(out=big[:, o, c0:c1], in_=acc[:, 0:c1 - c0])

        def cak_evac(o, acc, c0, c1):
            nc.vector.tensor_copy(out=cak[:, o, c0:c1], in_=acc[:, 0:c1 - c0])

        def ffn():
            """Fused w1 -> relu -> w2, L in 2 chunks of 384.

            h1 (2048 dims x 384 tokens, bf16) is staged into `big` (dead
            during FFN): slot idx -> big[:, idx % 8, 384*(idx//8):...].
            w2 then runs in two 4-output blocks (PSUM: 2 + 4 banks).
            """
            def h1slot(idx):
                b0 = 384 * (idx // DK)
                return big[:, idx % DK, b0:b0 + 384]

            with tc.tile_pool(name="ffn_ps", bufs=1, space="PSUM") as fps, \
                 tc.tile_pool(name="a1_ps", bufs=2, space="PSUM") as aps:
                for ci in range(2):
                    c0, c1 = CHN[ci], CHN[ci + 1]
                    for jb in range(2):
                        w1t = []
                        for k in range(DK):
                            w = wpool.tile([P, 1024], BF16, tag="w", name="w1t")
                            nc.sync.dma_start(
                                out=w,
                                in_=w1_d[l, k * P:(k + 1) * P,
                                         jb * 1024:(jb + 1) * 1024])
                            w1t.append(w)
                        for oo in range(DK):
                            a1 = aps.tile([P, 384], F32, tag="a1", name="a1")
                            for k in range(DK):
                                nc.tensor.matmul(
                                    a1, w1t[k][:, oo * P:(oo + 1) * P],
                                    hT[:, k, c0:c1],
                                    start=(k == 0), stop=(k == DK - 1))
                            nc.scalar.activation(h1slot(jb * DK + oo), a1,
                                                 AF.Relu)
                    for ob in range(2):
                        accs = [fps.tile([P, 384], F32, tag=f"facc{o}",
                                         name=f"facc{o}") for o in range(4)]
                        for k2 in range(2 * DK):
                            w2t = wpool.tile([P, 1024], BF16, tag="w",
                                             name="w2t")
                            nc.sync.dma_start(
                                out=w2t[:, 0:512],
                                in_=w2_d[l, k2 * P:(k2 + 1) * P,
                                         ob * 512:(ob + 1) * 512])
                            for o in range(4):
                                nc.tensor.matmul(
                                    accs[o], w2t[:, o * P:(o + 1) * P],
                                    h1slot(k2),
                                    start=(k2 == 0), stop=(k2 == 2 * DK - 1))
                        for o in range(4):
                            out_evac(ob * 4 + o, accs[o], c0, c1)

        # ---------------- resident V weights for layer 0 ----------------
        def load_wv(wvr, w_ap):
            for k in range(DK):
                nc.sync.dma_start(out=wvr[:, k, :], in_=w_ap[k * P:(k + 1) * P, :])

        # ---------------- layers ----------------
        for l in range(NLAYERS):
            load_wv(wvr_sa, wv_sa[l])
            # ---- self-attention ----
            layernorm()
            if l == 0:
                dump("dh1", hT, DK)
            vproj(wvr_sa, hT, vsa)
            proj(wqk_sa[l], OO, DK, 1024, lambda k: hT[:, k, :], qk_evac)
            if l == 0:
                dump("dqk", big, OO)
            attention(True, vsa, lambda j: big[:, 4 + j, :])
            if l == 0:
                dump("dctx", ctxT, QO)
            proj(wo_sa[l], OO, QO, 1024, lambda k: ctxT[:, k, :], out_evac)
            ar_issue()
            # AR1 shadow: CA k/v from mem (x-independent)
            load_wv(wvr_ca, wv_ca[l])
            proj(wk_ca[l], QO, DK, 512, lambda k: mem[:, k, :], cak_evac)
            vproj(wvr_ca, mem, vca)
            ar_accum()
            if l == 0:
                dump("dx1", xf, DK)

            # ---- cross-attention ----
            layernorm()
            proj(wq_ca[l], QO, DK, 512, lambda k: hT[:, k, :], qk_evac)
            if l == 0:
                dump("dcak", cak, QO)
            attention(False, vca, lambda j: cak[:, j, :])
            if l == 0:
                dump("dctx2", ctxT, QO)
            proj(wo_ca[l], OO, QO, 1024, lambda k: ctxT[:, k, :], out_evac)
            ar_issue()
            ar_accum()
            if l == 0:
                dump("dx2", xf, DK)

            # ---- FFN ----
            layernorm()
            ffn()
            ar_issue()
            ar_accum()
            if l == 0:
                dump("dx3", xf, DK)

        # ---------------- head ----------------
        for k in range(DK):
            nc.scalar.copy(hT[:, k, :], xf[:, k, :])

        def head_evac(o, acc, c0, c1):
            ev = evb.tile([P, 384], BF16, tag="ev", name="ev")
            nc.scalar.copy(ev[:, 0:c1 - c0], acc[:, 0:c1 - c0])
            nc.sync.dma_start(out=logits[o * P:(o + 1) * P, c0:c1],
                              in_=ev[:, 0:c1 - c0])

        proj(head_d, HEADO, DK, HEADO * P, lambda k: hT[:, k, :], head_evac)

    nc.finalize()
    return nc


# ---------------------------------------------------------------------------
# host side
# ---------------------------------------------------------------------------

def _pe_table(length, d):
    pos = np.arange(length, dtype=np.float32)[:, None]
    div = np.exp(np.arange(0, d, 2, dtype=np.float32) * (-np.log(10000.0) / d))
    ang = pos * div
    out = np.zeros((length, d), np.float32)
    out[:, 0::2] = np.sin(ang)
    out[:, 1::2] = np.cos(ang)
    return out


BF = ml_dtypes.bfloat16


def _bf(a):
    return np.ascontiguousarray(np.asarray(a, np.float32).astype(BF))


_NC_CACHE = {}
LAST_RESULT = {}


def kernel(**inputs):
    f32 = lambda a: np.asarray(a, dtype=np.float32)
    text = np.asarray(inputs["text"]).astype(np.int64)
    audio = np.asarray(inputs["audio"]).astype(np.int64)
    tl = np.asarray(inputs["text_len_batch"]).astype(np.int64)
    al = np.asarray(inputs["audio_len_batch"]).astype(np.int64)
    text_table = f32(inputs["text_table"])
    audio_table = f32(inputs["audio_table"])
    sa_in_w = f32(inputs["sa_in_w"])
    sa_out_w = f32(inputs["sa_out_w"])
    ca_in_w = f32(inputs["ca_in_w"])
    ca_out_w = f32(inputs["ca_out_w"])
    ffn_w1 = f32(inputs["ffn_w1"])
    ffn_w2 = f32(inputs["ffn_w2"])

    pe_t = _pe_table(Tt, D)
    pe_a = _pe_table(Ta, D)

    # per-TP-rank weight shards (shared by the 4 cores with the same rank)
    shard = []
    for r in range(2):
        sl = slice(512 * r, 512 * (r + 1))
        fl = slice(2048 * r, 2048 * (r + 1))
        wq, wk, wv = (sa_in_w[:NLAYERS, i * D:(i + 1) * D, :][:, sl]
                      for i in range(3))
        cq, ck, cv = (ca_in_w[:NLAYERS, i * D:(i + 1) * D, :][:, sl]
                      for i in range(3))
        hw = np.zeros((HEADO * P, D), np.float32)
        hw[0:513] = audio_table[513 * r:513 * (r + 1)]
        shard.append({
            "wqk_sa": _bf(np.concatenate([wq, wk], axis=1).transpose(0, 2, 1)),
            "wv_sa": _bf(wv.transpose(0, 2, 1)),
            "wo_sa": _bf(sa_out_w[:NLAYERS, :, sl].transpose(0, 2, 1)),
            "wq_ca": _bf(cq.transpose(0, 2, 1)),
            "wk_ca": _bf(ck.transpose(0, 2, 1)),
            "wv_ca": _bf(cv.transpose(0, 2, 1)),
            "wo_ca": _bf(ca_out_w[:NLAYERS, :, sl].transpose(0, 2, 1)),
            "w1T": _bf(ffn_w1[:NLAYERS, fl, :].transpose(0, 2, 1)),
            "w2T": _bf(ffn_w2[:NLAYERS, :, fl].transpose(0, 2, 1)),
            "headT": _bf(hw.T),
        })

    in_maps = []
    for c in range(8):
        p, r = c // 2, c % 2
        tlb, alb = int(tl[p]), int(al[p])
        il = tlb + alb

        emb = np.zeros((L, D), np.float32)
        emb[:tlb] = text_table[text[p, :tlb]] + pe_t[:tlb]
        emb[tlb:il] = audio_table[audio[p, :alb]] + pe_a[:alb]
        x0 = np.ascontiguousarray(emb.T)

        kk = np.arange(L)
        allowed = ((kk[None, :] < tlb)
                   | ((kk[None, :] <= kk[:, None]) & (kk[:, None] < il)))
        # maskT[k_part, tk*L + q] = allowed(q, tk*128 + k_part)
        mt = allowed.T.reshape(NT, P, L).transpose(1, 0, 2).reshape(P, NT * L)

        m = {"x0": x0, "mem0": _bf(x0), "maskT": _bf(mt)}
        m.update(shard[r])
        in_maps.append(m)

    key = (NLAYERS, os.environ.get("KERNEL_DEBUG", "0"))
    if _NC_CACHE.get("key") != key:
        _NC_CACHE["nc"] = _build_nc()
        _NC_CACHE["key"] = key
    nc = _NC_CACHE["nc"]
    trace = bool(int(os.environ.get("KERNEL_TRACE", "0")))
    r = run_bass_kernel_spmd(nc, in_maps, core_ids=list(range(8)), trace=trace)
    LAST_RESULT["r"] = r
    res = r.results

    out = np.empty((B, L, VA), np.float32)
    for p in range(B):
        ev = np.asarray(res[2 * p]["logits"], dtype=np.float32)
        od = np.asarray(res[2 * p + 1]["logits"], dtype=np.float32)
        out[p] = np.concatenate([ev[0:513], od[0:513]], axis=0).T
    return out


# revision 28
# speedup vs baseline: 1.7852x; 1.1379x over previous
"""Trainium2 Bass kernel for nn_AutoRegressive (dense transformer decoder), v2.

Model: B=4 packed text+audio sequences, L=768, D=1024, 16 heads, DFF=4096,
6 norm-first decoder layers (self-attn w/ prefix-LM mask, cross-attn to the
packed embedding, FFN), weight-tied audio head. fp32 inputs/outputs.

Sharding: DP4 x TP2 over 8 cores. Core pair (2i, 2i+1) owns batch item i;
within a pair the 16 heads split 8+8 and DFF splits 2048+2048. Three pair-
AllReduces per layer; CA K/V projections are scheduled into AR shadows.

v2 design vs baseline:
- all weights pre-transposed + bf16 on host (no PE transposes, no ACT copies)
- activations bf16 (residual stream f32), matmuls bf16 @ 1 cyc/row
- embedding + positional encoding gathered/packed on host -> x0 DMA
- V projected token-major directly (stationary = activation tile), so AV
  stationary needs no on-device transpose; ones column gives softmax denom
- SA score tiles that are fully masked are skipped (12 of 18 (tk,chunk))
- mask applied multiplicatively AFTER exp (bf16 x bf16 -> 4x DVE rate)
- projections stream weights in L-chunks of 384 with k-outer/o-inner loops,
  FFN is fused (w1 block -> relu -> w2 block) to bound PSUM/SBUF residency
"""
import os
import numpy as np
import ml_dtypes

import concourse.bass as bass
from concourse import bacc
import concourse.mybir as mybir
import concourse.tile as tile
from concourse.bass_utils import run_bass_kernel_spmd

F32 = mybir.dt.float32
BF16 = mybir.dt.bfloat16
AF = mybir.ActivationFunctionType
OP = mybir.AluOpType

B, Tt, Ta, L, D, H, DH, DFF, NL = 4, 128, 640, 768, 1024, 16, 64, 4096, 6
VT, VA = 256, 1026
NLAYERS = int(os.environ.get("KERNEL_NL", str(NL)))
P = 128
NT = L // P          # 6 sequence tiles
DK = D // P          # 8 feature tiles
QO = 4               # local q/k/ctx tiles (512 dims)
OO = 8               # output tiles of D
HLOC = 8             # local heads
HEADO = 5            # head out-tiles (640-row padded vocab slab)
CHN = (0, 384, 768)  # L chunks for streamed projections
# SA (tk allowed) per 256-wide q chunk c (q tiles 2c, 2c+1)
SA_SET = {0: (0, 1), 1: (0, 1, 2, 3), 2: (0, 1, 2, 3, 4, 5)}


def _build_nc():
    nc = bacc.Bacc(None)
    rsem = nc.alloc_semaphore("ar_rsem")
    lsem = nc.alloc_semaphore("ar_lsem")

    x0_d = nc.declare_dram_parameter("x0", [D, L], F32, isOutput=False)
    mem_d = nc.declare_dram_parameter("mem0", [D, L], BF16, isOutput=False)
    mask_d = nc.declare_dram_parameter("maskT", [P, NT * L], BF16, isOutput=False)
    wqk_sa = nc.declare_dram_parameter("wqk_sa", [NLAYERS, D, 1024], BF16, isOutput=False)
    wv_sa = nc.declare_dram_parameter("wv_sa", [NLAYERS, D, 512], BF16, isOutput=False)
    wo_sa = nc.declare_dram_parameter("wo_sa", [NLAYERS, 512, D], BF16, isOutput=False)
    wq_ca = nc.declare_dram_parameter("wq_ca", [NLAYERS, D, 512], BF16, isOutput=False)
    wk_ca = nc.declare_dram_parameter("wk_ca", [NLAYERS, D, 512], BF16, isOutput=False)
    wv_ca = nc.declare_dram_parameter("wv_ca", [NLAYERS, D, 512], BF16, isOutput=False)
    wo_ca = nc.declare_dram_parameter("wo_ca", [NLAYERS, 512, D], BF16, isOutput=False)
    w1_d = nc.declare_dram_parameter("w1T", [NLAYERS, D, 2048], BF16, isOutput=False)
    w2_d = nc.declare_dram_parameter("w2T", [NLAYERS, 2048, D], BF16, isOutput=False)
    head_d = nc.declare_dram_parameter("headT", [D, HEADO * P], BF16, isOutput=False)
    logits = nc.declare_dram_parameter("logits", [HEADO * P, L], BF16, isOutput=True)

    DBG = bool(int(os.environ.get("KERNEL_DEBUG", "0")))
    dbg = {}
    if DBG:
        for nm, shp, dt in [("dh1", [D, L], BF16), ("dqk", [1024, L], BF16),
                            ("dctx", [512, L], BF16), ("dx1", [D, L], F32),
                            ("dcak", [512, L], BF16), ("dctx2", [512, L], BF16),
                            ("dx2", [D, L], F32), ("dx3", [D, L], F32)]:
            dbg[nm] = nc.declare_dram_parameter(nm, shp, dt, isOutput=True)

    GROUPS = [[0, 1], [2, 3], [4, 5], [6, 7]]

    from contextlib import ExitStack
    with tile.TileContext(nc) as tc, ExitStack() as S:
        state = S.enter_context(tc.tile_pool(name="state", bufs=1))
        wpool = S.enter_context(tc.tile_pool(name="wpool", bufs=12))
        evb = S.enter_context(tc.tile_pool(name="evb", bufs=2))
        prb = S.enter_context(tc.tile_pool(name="prb", bufs=3))
        xsqp = S.enter_context(tc.tile_pool(name="xsqp", bufs=1))
        tmpp = S.enter_context(tc.tile_pool(name="tmpp", bufs=2))
        bcp = S.enter_context(tc.tile_pool(name="bcp", bufs=1))
        bcb = S.enter_context(tc.tile_pool(name="bcb", bufs=2))
        invp = S.enter_context(tc.tile_pool(name="invp", bufs=1))

        xf = state.tile([P, DK, L], F32)
        mem = state.tile([P, DK, L], BF16)
        hT = state.tile([P, DK, L], BF16)
        big = state.tile([P, OO, L], BF16)       # q(0:4)+k(4:8); FFN h1
        ctxT = state.tile([P, QO, L], BF16)
        vtok = state.tile([P, NT, HLOC, 66], BF16)
        mk = state.tile([P, NT, L], BF16)
        pbuf = state.tile([P, DK, L], BF16)      # my AR partial (send src)
        rbuf = state.tile([P, 2, DK, L], BF16)   # peer partial (recv, x2 parity)
        wvr_sa = state.tile([P, DK, 512], BF16)  # resident v weights
        wvr_ca = state.tile([P, DK, 512], BF16)
        onesb = state.tile([P, 1], BF16)
        epst = state.tile([1, 1], F32)
        mu_s = state.tile([1, L], F32)
        var_s = state.tile([1, L], F32)
        sd_s = state.tile([1, L], F32)
        sdb_s = state.tile([1, L], BF16)

        nc.vector.memset(onesb, 1.0)
        nc.vector.memset(epst, 1e-5)
        nc.vector.memset(vtok[:, :, :, 64:65], 1.0)
        for k in range(DK):
            nc.sync.dma_start(out=xf[:, k, :], in_=x0_d[k * P:(k + 1) * P, :])
            nc.sync.dma_start(out=mem[:, k, :], in_=mem_d[k * P:(k + 1) * P, :])
        for t in range(NT):
            nc.sync.dma_start(out=mk[:, t, :], in_=mask_d[:, t * L:(t + 1) * L])
        # all peers must be in-kernel before any remote_dma into their SBUF
        nc.gpsimd.bir_kernel_barrier_wait(GROUPS)

        def dump(nm, tile_ap, n, off=0):
            if not DBG:
                return
            d = dbg[nm]
            for o in range(n):
                nc.sync.dma_start(out=d[o * P:(o + 1) * P, :],
                                  in_=tile_ap[:, off + o, :])

        # ---------------- helpers ----------------
        def layernorm():
            """LN over partition dim of xf -> hT (no affine; w=1, b=0)."""
            with tc.tile_pool(name="ln_ps", bufs=1, space="PSUM") as lps:
                s12 = lps.tile([33, L], F32)
                for k in range(DK):
                    xsq = xsqp.tile([P, 2, L], BF16, tag="xsq", name="xsq")
                    nc.scalar.copy(xsq[:, 0, :], xf[:, k, :])
                    nc.scalar.activation(xsq[:, 1, :], xf[:, k, :], AF.Square)
                    st, sp = (k == 0), (k == DK - 1)
                    for c0, c1 in ((0, 512), (512, L)):
                        nc.tensor.matmul(s12[0:1, c0:c1], onesb, xsq[:, 0, c0:c1],
                                         start=st, stop=sp)
                        nc.tensor.matmul(s12[32:33, c0:c1], onesb, xsq[:, 1, c0:c1],
                                         start=st, stop=sp)
                nc.vector.tensor_scalar_mul(mu_s, s12[0:1, :], 1.0 / D)
                nc.vector.tensor_mul(out=var_s, in0=mu_s, in1=mu_s)
                nc.vector.scalar_tensor_tensor(
                    out=var_s, in0=s12[32:33, :], scalar=1.0 / D,
                    in1=var_s, op0=OP.mult, op1=OP.subtract)
            nc.scalar.activation(sd_s, var_s, AF.Sqrt, bias=epst[0:1, 0:1])
            nc.vector.reciprocal_approx_fast(out=var_s, in_=sd_s)
            nc.scalar.copy(sdb_s, var_s)
            mub = bcp.tile([P, L], F32, tag="mub", name="mub")
            nc.gpsimd.partition_broadcast(mub, mu_s[0:1, :])
            rb = bcb.tile([P, L], BF16, tag="rb", name="rb")
            nc.gpsimd.partition_broadcast(rb, sdb_s[0:1, :])
            for k in range(DK):
                t = tmpp.tile([P, L], BF16, tag="lt", name="lt")
                nc.vector.tensor_tensor(out=t, in0=xf[:, k, :], in1=mub,
                                        op=OP.subtract)
                nc.vector.tensor_mul(out=hT[:, k, :], in0=t, in1=rb)

        def proj(w_ap, n_o, n_k, wcols, rhs_fn, out_fn):
            """Streamed projection: out[o] = sum_k W^T[k]-tile @ rhs(k).

            k-outer / o-inner, full-L accumulators; the two L chunks share
            one stationary load. w_ap: [Din, wcols] dram slice, rhs_fn(k) ->
            bf16 AP [128, L]; out_fn(o, acc) with acc [128, L] f32 psum.
            """
            with tc.tile_pool(name="pj_ps", bufs=1, space="PSUM") as pps:
                for ob0 in range(0, n_o, 4):
                    obn = min(4, n_o - ob0)
                    accs = [pps.tile([P, L], F32, tag=f"pacc{o}",
                                     name=f"pacc{o}") for o in range(obn)]
                    for k in range(n_k):
                        wt = wpool.tile([P, 1024], BF16, tag="w", name="wt")
                        nc.sync.dma_start(
                            out=wt[:, 0:obn * P],
                            in_=w_ap[k * P:(k + 1) * P,
                                     ob0 * P:(ob0 + obn) * P])
                        rhs = rhs_fn(k)
                        st, sp = (k == 0), (k == n_k - 1)
                        for o in range(obn):
                            w_sl = wt[:, o * P:(o + 1) * P]
                            nc.tensor.matmul(accs[o][:, 0:512], w_sl,
                                             rhs[:, 0:512], start=st, stop=sp)
                            nc.tensor.matmul(accs[o][:, 512:L], w_sl,
                                             rhs[:, 512:L], start=st, stop=sp)
                    for o in range(obn):
                        out_fn(ob0 + o, accs[o])

        def vproj(wvr, src, vdst):
            """Token-major V: vdst[:, t, h, 0:64] = (src^T W_v^T)[t-tile]."""
            with tc.tile_pool(name="v_ps", bufs=2, space="PSUM") as vps:
                for t in range(NT):
                    vacc = vps.tile([P, DK, 64], F32, tag="vacc", name="vacc")
                    for k in range(DK):
                        nc.tensor.matmul(vacc[:, :, :],
                                         src[:, k, t * P:(t + 1) * P],
                                         wvr[:, k, :],
                                         start=(k == 0), stop=(k == DK - 1))
                    nc.vector.tensor_copy(out=vdst[:, t, :, 0:64],
                                          in_=vacc[:, :, :])

        def attention(masked, vtok, kt_fn):
            """softmax((big q)^T k / 8 + mask) @ v -> ctxT (denom via ones col).

            Per (head, key-tile): one stationary load for scores over the
            whole allowed q range, one exp, one mask multiply; AV shares the
            vtok stationary across both 512-boundary chunks. For SA, key
            tile tk only reaches queries q >= qlo(tk) (prefix text keys are
            all in tk 0), so higher tiles start at 256/512.
            """
            with tc.tile_pool(name="at_sps", bufs=2, space="PSUM") as sps, \
                 tc.tile_pool(name="at_cps", bufs=2, space="PSUM") as cps:
                for h in range(HLOC):
                    j, hb = h // 2, 64 * (h % 2)
                    ctx = cps.tile([P, L], F32, tag="ctx", name="ctx")
                    for tk in range(NT):
                        qlo = (0, 0, 256, 256, 512, 512)[tk] if masked else 0
                        sc = sps.tile([P, L], F32, tag="sc", name="sc")
                        kT = kt_fn(j)[hb:hb + 64, tk * P:(tk + 1) * P]
                        for c0, c1 in (((qlo, 512), (512, L)) if qlo < 512
                                       else ((512, L),)):
                            nc.tensor.matmul(sc[:, c0:c1], kT,
                                             big[hb:hb + 64, j, c0:c1],
                                             start=True, stop=True)
                        pr = prb.tile([P, L], BF16, tag="pr", name="pr")
                        nc.scalar.activation(pr[:, qlo:L], sc[:, qlo:L],
                                             AF.Exp, scale=0.125)
                        if masked:
                            nc.vector.tensor_mul(out=pr[:, qlo:L],
                                                 in0=pr[:, qlo:L],
                                                 in1=mk[:, tk, qlo:L])
                        for c0, c1 in (((qlo, 512), (512, L)) if qlo < 512
                                       else ((512, L),)):
                            nc.tensor.matmul(ctx[0:65, c0:c1],
                                             vtok[:, tk, h, 0:65],
                                             pr[:, c0:c1],
                                             start=(tk == 0),
                                             stop=(tk == NT - 1))
                    den = invp.tile([1, L], F32, tag="den", name="den")
                    nc.scalar.copy(den, ctx[64:65, :])
                    inv1 = invp.tile([1, L], F32, tag="inv", name="inv1")
                    nc.vector.reciprocal_approx_fast(out=inv1, in_=den)
                    invb = bcp.tile([P, L], F32, tag="invb", name="invb")
                    nc.gpsimd.partition_broadcast(invb, inv1[0:1, :])
                    nc.vector.tensor_mul(out=ctxT[hb:hb + 64, j, :],
                                         in0=ctx[0:64, :], in1=invb[0:64, :])

        def out_evac(o, acc):
            nc.scalar.copy(pbuf[:, o, :], acc)

        def out_evac_c(o, acc, c0, c1):
            nc.scalar.copy(pbuf[:, o, c0:c1], acc[:, 0:c1 - c0])

        ar_n = [0]

        def ar_issue():
            """Pair AllReduce via direct SBUF->SBUF remote DMA (peer = tpb^1).

            Each o-row goes out on its own engine-pair slot; parity double-
            buffers rbuf so a core at most one AR ahead never overwrites
            data its peer is still reading.
            """
            par = ar_n[0] % 2
            for o in range(DK):
                rdests = [None] * 8
                rdests[o] = (0, 1)
                nc.gpsimd.remote_dma_broadcast(
                    out_ap=rbuf[:, par, o, :], in_ap=pbuf[:, o, :],
                    remote_sem=rsem, local_sem=lsem, rdests=rdests)
            nc.gpsimd.trigger_dma(count=None)

        def ar_accum():
            n = ar_n[0]
            par = n % 2
            nc.vector.wait_ge(rsem, 16 * (n + 1))   # peer data landed
            nc.scalar.wait_ge(lsem, 128 * (n + 1))  # my sends drained (pbuf WAR)
            for o in range(DK):
                nc.vector.tensor_tensor(out=xf[:, o, :], in0=xf[:, o, :],
                                        in1=pbuf[:, o, :], op=OP.add)
                nc.vector.tensor_tensor(out=xf[:, o, :], in0=xf[:, o, :],
                                        in1=rbuf[:, par, o, :], op=OP.add)
            ar_n[0] = n + 1

        def qk_evac(o, acc):
            nc.vector.tensor_copy(out=big[:, o, :], in_=acc)

        def cak_evac(o, acc):
            nc.scalar.copy(big[:, 4 + o, :], acc)

        def ffn():
            """Fused w1 -> relu -> w2, L in 2 chunks of 384.

            h1 (2048 dims x 384 tokens, bf16) is staged into `big` (dead
            during FFN): slot idx -> big[:, idx % 8, 384*(idx//8):...].
            w2 then runs in two 4-output blocks (PSUM: 2 + 4 banks).
            """
            def h1slot(idx):
                b0 = 384 * (idx // DK)
                return big[:, idx % DK, b0:b0 + 384]

            with tc.tile_pool(name="ffn_ps", bufs=1, space="PSUM") as fps, \
                 tc.tile_pool(name="a1_ps", bufs=2, space="PSUM") as aps:
                for ci in range(2):
                    c0, c1 = CHN[ci], CHN[ci + 1]
                    for jb in range(2):
                        w1t = []
                        for k in range(DK):
                            w = wpool.tile([P, 1024], BF16, tag="w", name="w1t")
                            nc.sync.dma_start(
                                out=w,
                                in_=w1_d[l, k * P:(k + 1) * P,
                                         jb * 1024:(jb + 1) * 1024])
                            w1t.append(w)
                        for oo in range(DK):
                            a1 = aps.tile([P, 384], F32, tag="a1", name="a1")
                            for k in range(DK):
                                nc.tensor.matmul(
                                    a1, w1t[k][:, oo * P:(oo + 1) * P],
                                    hT[:, k, c0:c1],
                                    start=(k == 0), stop=(k == DK - 1))
                            nc.scalar.activation(h1slot(jb * DK + oo), a1,
                                                 AF.Relu)
                    for ob in range(2):
                        accs = [fps.tile([P, 384], F32, tag=f"facc{o}",
                                         name=f"facc{o}") for o in range(4)]
                        for k2 in range(2 * DK):
                            w2t = wpool.tile([P, 1024], BF16, tag="w",
                                             name="w2t")
                            nc.sync.dma_start(
                                out=w2t[:, 0:512],
                                in_=w2_d[l, k2 * P:(k2 + 1) * P,
                                         ob * 512:(ob + 1) * 512])
                            for o in range(4):
                                nc.tensor.matmul(
                                    accs[o], w2t[:, o * P:(o + 1) * P],
                                    h1slot(k2),
                                    start=(k2 == 0), stop=(k2 == 2 * DK - 1))
                        for o in range(4):
                            out_evac_c(ob * 4 + o, accs[o], c0, c1)

        # ---------------- resident V weights for layer 0 ----------------
        def load_wv(wvr, w_ap):
            for k in range(DK):
                nc.sync.dma_start(out=wvr[:, k, :], in_=w_ap[k * P:(k + 1) * P, :])

        # ---------------- layers ----------------
        for l in range(NLAYERS):
            load_wv(wvr_sa, wv_sa[l])
            # ---- self-attention ----
            layernorm()
            if l == 0:
                dump("dh1", hT, DK)
            vproj(wvr_sa, hT, vtok)
            proj(wqk_sa[l], OO, DK, 1024, lambda k: hT[:, k, :], qk_evac)
            if l == 0:
                dump("dqk", big, OO)
            attention(True, vtok, lambda j: big[:, 4 + j, :])
            if l == 0:
                dump("dctx", ctxT, QO)
            proj(wo_sa[l], OO, QO, 1024, lambda k: ctxT[:, k, :], out_evac)
            ar_issue()
            # AR1 shadow: CA k/v from mem (x-independent); k reuses big[4:8],
            # v reuses vtok (SA attention is done with both by this point)
            load_wv(wvr_ca, wv_ca[l])
            proj(wk_ca[l], QO, DK, 512, lambda k: mem[:, k, :], cak_evac)
            vproj(wvr_ca, mem, vtok)
            ar_accum()
            if l == 0:
                dump("dx1", xf, DK)

            # ---- cross-attention ----
            layernorm()
            proj(wq_ca[l], QO, DK, 512, lambda k: hT[:, k, :], qk_evac)
            if l == 0:
                dump("dcak", big, QO, off=4)
            attention(False, vtok, lambda j: big[:, 4 + j, :])
            if l == 0:
                dump("dctx2", ctxT, QO)
            proj(wo_ca[l], OO, QO, 1024, lambda k: ctxT[:, k, :], out_evac)
            ar_issue()
            ar_accum()
            if l == 0:
                dump("dx2", xf, DK)

            # ---- FFN ----
            layernorm()
            ffn()
            ar_issue()
            ar_accum()
            if l == 0:
                dump("dx3", xf, DK)

        # ---------------- head ----------------
        for k in range(DK):
            nc.scalar.copy(hT[:, k, :], xf[:, k, :])

        def head_evac(o, acc):
            ev = evb.tile([P, L], BF16, tag="ev", name="ev")
            nc.scalar.copy(ev, acc)
            nc.sync.dma_start(out=logits[o * P:(o + 1) * P, :], in_=ev)

        proj(head_d, HEADO, DK, HEADO * P, lambda k: hT[:, k, :], head_evac)

    nc.finalize()
    return nc


# ---------------------------------------------------------------------------
# host side
# ---------------------------------------------------------------------------

def _pe_table(length, d):
    pos = np.arange(length, dtype=np.float32)[:, None]
    div = np.exp(np.arange(0, d, 2, dtype=np.float32) * (-np.log(10000.0) / d))
    ang = pos * div
    out = np.zeros((length, d), np.float32)
    out[:, 0::2] = np.sin(ang)
    out[:, 1::2] = np.cos(ang)
    return out


BF = ml_dtypes.bfloat16


def _bf(a):
    return np.ascontiguousarray(np.asarray(a, np.float32).astype(BF))


_NC_CACHE = {}
LAST_RESULT = {}


def kernel(**inputs):
    f32 = lambda a: np.asarray(a, dtype=np.float32)
    text = np.asarray(inputs["text"]).astype(np.int64)
    audio = np.asarray(inputs["audio"]).astype(np.int64)
    tl = np.asarray(inputs["text_len_batch"]).astype(np.int64)
    al = np.asarray(inputs["audio_len_batch"]).astype(np.int64)
    text_table = f32(inputs["text_table"])
    audio_table = f32(inputs["audio_table"])
    sa_in_w = f32(inputs["sa_in_w"])
    sa_out_w = f32(inputs["sa_out_w"])
    ca_in_w = f32(inputs["ca_in_w"])
    ca_out_w = f32(inputs["ca_out_w"])
    ffn_w1 = f32(inputs["ffn_w1"])
    ffn_w2 = f32(inputs["ffn_w2"])

    pe_t = _pe_table(Tt, D)
    pe_a = _pe_table(Ta, D)

    # per-TP-rank weight shards (shared by the 4 cores with the same rank)
    shard = []
    for r in range(2):
        sl = slice(512 * r, 512 * (r + 1))
        fl = slice(2048 * r, 2048 * (r + 1))
        wq, wk, wv = (sa_in_w[:NLAYERS, i * D:(i + 1) * D, :][:, sl]
                      for i in range(3))
        cq, ck, cv = (ca_in_w[:NLAYERS, i * D:(i + 1) * D, :][:, sl]
                      for i in range(3))
        hw = np.zeros((HEADO * P, D), np.float32)
        hw[0:513] = audio_table[513 * r:513 * (r + 1)]
        shard.append({
            "wqk_sa": _bf(np.concatenate([wq, wk], axis=1).transpose(0, 2, 1)),
            "wv_sa": _bf(wv.transpose(0, 2, 1)),
            "wo_sa": _bf(sa_out_w[:NLAYERS, :, sl].transpose(0, 2, 1)),
            "wq_ca": _bf(cq.transpose(0, 2, 1)),
            "wk_ca": _bf(ck.transpose(0, 2, 1)),
            "wv_ca": _bf(cv.transpose(0, 2, 1)),
            "wo_ca": _bf(ca_out_w[:NLAYERS, :, sl].transpose(0, 2, 1)),
            "w1T": _bf(ffn_w1[:NLAYERS, fl, :].transpose(0, 2, 1)),
            "w2T": _bf(ffn_w2[:NLAYERS, :, fl].transpose(0, 2, 1)),
            "headT": _bf(hw.T),
        })

    in_maps = []
    for c in range(8):
        p, r = c // 2, c % 2
        tlb, alb = int(tl[p]), int(al[p])
        il = tlb + alb

        emb = np.zeros((L, D), np.float32)
        emb[:tlb] = text_table[text[p, :tlb]] + pe_t[:tlb]
        emb[tlb:il] = audio_table[audio[p, :alb]] + pe_a[:alb]
        x0 = np.ascontiguousarray(emb.T)

        kk = np.arange(L)
        allowed = ((kk[None, :] < tlb)
                   | ((kk[None, :] <= kk[:, None]) & (kk[:, None] < il)))
        # maskT[k_part, tk*L + q] = allowed(q, tk*128 + k_part)
        mt = allowed.T.reshape(NT, P, L).transpose(1, 0, 2).reshape(P, NT * L)

        m = {"x0": x0, "mem0": _bf(x0), "maskT": _bf(mt)}
        m.update(shard[r])
        in_maps.append(m)

    key = (NLAYERS, os.environ.get("KERNEL_DEBUG", "0"))
    if _NC_CACHE.get("key") != key:
        _NC_CACHE["nc"] = _build_nc()
        _NC_CACHE["key"] = key
    nc = _NC_CACHE["nc"]
    trace = bool(int(os.environ.get("KERNEL_TRACE", "0")))
    r = run_bass_kernel_spmd(nc, in_maps, core_ids=list(range(8)), trace=trace)
    LAST_RESULT["r"] = r
    res = r.results

    out = np.empty((B, L, VA), np.float32)
    for p in range(B):
        ev = np.asarray(res[2 * p]["logits"], dtype=np.float32)
        od = np.asarray(res[2 * p + 1]["logits"], dtype=np.float32)
        out[p] = np.concatenate([ev[0:513], od[0:513]], axis=0).T
    return out
